# revision 18
# baseline (speedup 1.0000x reference)
"""MAM dense kernel for Trainium2 (8 NeuronCores).

C[n,j] = max_k(x[n,k]*w[j,k]) + min_k(x[n,k]*w[j,k]) + bias[j]

Strategy: tensor-parallel over out_features (32 j per core), batch rows on
SBUF partitions (16 tiles of 128 rows; every core reads all of x). Each
core's weight slice (32x512) arrives pre-replicated across the 128
partitions from the host. Per x tile the DVE multiplies x (broadcast
along j via a 0-stride AP dim) against the replicated weights into a
[128, 32*512] product buffer, then grouped tensor_reduce computes max and
min over k per output feature. max+min summed on device; bias added on
host.

Raw Bass (manual semaphores): this toolchain's walrus allows at most one
attached sync-wait per compute instruction, which rules out the Tile
scheduler; standalone wait_ge instructions are used instead. Double
buffered x loads and output stores overlap DMA with DVE compute.
"""

import sys

sys.path.insert(0, "/opt/trn_rl_repo")

import numpy as np

import concourse.bass as bass
import concourse.mybir as mybir
from concourse.bass_utils import run_bass_kernel_spmd

N = 2048
IN_F = 512
OUT_F = 256
NCORES = 8
JS = OUT_F // NCORES          # 32 output features per core
NT = N // 128                 # 16 row tiles
DT = mybir.dt.float32
F32 = mybir.dt.float32

_cached = {}
TRACE = False
LAST_EXEC_NS = None


def _build_nc():
    nc = bass.Bass()
    x_in = nc.declare_dram_parameter("x", [N, IN_F], DT, isOutput=False)
    w_in = nc.declare_dram_parameter("w_rep", [128, JS * IN_F], DT, isOutput=False)
    out = nc.declare_dram_parameter("out", [N, JS], F32, isOutput=True)

    x_t = x_in.rearrange("(t p) k -> t p k", p=128)
    out_t = out.rearrange("(t p) j -> t p j", p=128)

    with (
        nc.sbuf_tensor([128, JS * IN_F], DT) as wt,
        nc.sbuf_tensor([128, JS * IN_F], DT) as prod,
        nc.sbuf_tensor([128, 2 * IN_F], DT) as xt,      # ping-pong x tiles
        nc.sbuf_tensor([128, 2 * JS], F32) as ot,        # ping-pong outputs
        nc.sbuf_tensor([128, 2 * JS], F32) as mx,        # max | min accums
        nc.semaphore("load_sem0") as load_sem0,
        nc.semaphore("load_sem1") as load_sem1,
        nc.semaphore("w_sem") as w_sem,
        nc.semaphore("store_sem0") as store_sem0,
        nc.semaphore("store_sem1") as store_sem1,
        nc.semaphore("v_sem") as v_sem,
        nc.Block() as block,
    ):

        @block.sync
        def _(sync):
            # weights + first two x tiles
            for c in range(0, JS * IN_F, IN_F):
                sync.dma_start(wt[:, c : c + IN_F], w_in[:, c : c + IN_F]).then_inc(
                    w_sem, 16
                )
            sync.dma_start(xt[:, 0:IN_F], x_t[0]).then_inc(load_sem0, 16)
            sync.dma_start(xt[:, IN_F : 2 * IN_F], x_t[1]).then_inc(load_sem1, 16)
            for i in range(NT):
                # wait for DVE to finish tile i (2 incs per tile)
                sync.wait_ge(v_sem, 2 * i + 2)
                b = (i % 2) * JS
                ssem = store_sem0 if i % 2 == 0 else store_sem1
                sync.dma_start(out_t[i], ot[:, b : b + JS]).then_inc(ssem, 16)
                if i + 2 < NT:
                    xb = (i % 2) * IN_F
                    lsem = load_sem0 if i % 2 == 0 else load_sem1
                    sync.dma_start(xt[:, xb : xb + IN_F], x_t[i + 2]).then_inc(
                        lsem, 16
                    )

        @block.vector
        def _(vector):
            vector.wait_ge(w_sem, 16 * JS)
            prod3 = prod[:].rearrange("p (j k) -> p j k", k=IN_F)
            wt3 = wt[:].rearrange("p (j k) -> p j k", k=IN_F)
            for i in range(NT):
                # x tile i loaded (parity semaphore identifies the slot)
                vector.wait_ge(
                    load_sem0 if i % 2 == 0 else load_sem1, 16 * (i // 2 + 1)
                )
                xb = (i % 2) * IN_F
                x_b = xt[:, xb : xb + IN_F].unsqueeze(1).broadcast_to(
                    (128, JS, IN_F)
                )
                nc.vector.tensor_tensor(
                    out=prod3, in0=x_b, in1=wt3, op=mybir.AluOpType.mult
                ).then_inc(v_sem, 1)
                nc.vector.tensor_reduce(
                    out=mx[:, 0:JS], in_=prod3, axis=mybir.AxisListType.X,
                    op=mybir.AluOpType.max,
                )
                nc.vector.tensor_reduce(
                    out=mx[:, JS : 2 * JS], in_=prod3, axis=mybir.AxisListType.X,
                    op=mybir.AluOpType.min,
                )
                if i >= 2:
                    # output slot i%2 free once store of tile i-2 completed
                    vector.wait_ge(
                        store_sem0 if i % 2 == 0 else store_sem1, 16 * (i // 2)
                    )
                b = (i % 2) * JS
                nc.vector.tensor_tensor(
                    out=ot[:, b : b + JS], in0=mx[:, 0:JS], in1=mx[:, JS : 2 * JS],
                    op=mybir.AluOpType.add,
                )
                # DVE write-acks are pipelined: the retire (and sem inc) of a
                # DVE op can precede its SBUF bytes landing. The next DVE op
                # only issues after the pipe drains, so carrying the inc on a
                # dummy op guarantees the store DMA reads settled data.
                nc.vector.tensor_copy(prod[:, 0:2], mx[:, 0:2]).then_inc(v_sem, 1)

    return nc


def kernel(x: np.ndarray, weight: np.ndarray, bias: np.ndarray) -> np.ndarray:
    if "nc" not in _cached:
        _cached["nc"] = _build_nc()
    nc = _cached["nc"]

    x = np.ascontiguousarray(x, dtype=np.float32)
    weight = np.asarray(weight, dtype=np.float32)

    in_maps = []
    for c in range(NCORES):
        w_slice = weight[c * JS : (c + 1) * JS, :].reshape(1, JS * IN_F)
        w_rep = np.ascontiguousarray(np.broadcast_to(w_slice, (128, JS * IN_F)))
        in_maps.append({"x": x, "w_rep": w_rep})

    res = run_bass_kernel_spmd(nc, in_maps, list(range(NCORES)), trace=TRACE)
    global LAST_EXEC_NS
    LAST_EXEC_NS = getattr(res, 'exec_time_ns', None)
    outs = [np.asarray(res.results[c]["out"]) for c in range(NCORES)]
    full = np.concatenate(outs, axis=1)
    return (full + np.asarray(bias, dtype=np.float32)[None, :]).astype(np.float32)
